# revision 117
# baseline (speedup 1.0000x reference)
"""Trainium2 Bass kernel for nn_CAMD_9990093930844 (sparse_attention).

Math: the reference computes, per modality m,
    out_m[i, :] = Q[i] @ S_m(t1[i]) ,  S_m(t) = sum_{j: t2_m[j] <= t} K_m[j] (x) V_m[j]
and returns (sum_m out_m)[:, :2].  Only V[:, :2] matters, so this is
    out[i, v] = sum_m sum_{j: t2_m[j] <= t1[i]} (Q[i] . K_m[j]) * V_m[j, v]

Both t1 and t2_m are sorted, so the rank deviation |p_m[i] - i| (p =
searchsorted) is bounded (~90 for this data).  Each 128-query chunk b needs
  - an unconditional prefix state over key chunks [0, b-1)
  - a masked local attention over key chunks {b-1, b, b+1}

Sharding: 8 cores = 4 modalities x 2 query halves; host does the final sum
over modalities + concat (tiny).  33-slot zero/sentinel-padded local key
buffer per core; uniform SPMD program.

Device-side structure (driven by TRN2 cost/validity constraints):
  - fp16 matmul operands everywhere (1 cycle/column, no 32-bit mixing);
    biases folded via ones-rows, so PSUM drains are pure Relu/Copy.
  - The LAST MLP layer is never materialized: with M = W6'@Wq3'^T (host),
    scores = hbk'^T (M hbq') via one extra fp16 "P_q" layer on Q, and
    prefix states live in the G-domain (G = hb'^T V), transformed once by
    Mq = M^T at the end.  This removes the K-side C-layer entirely.
  - Output accumulation is token-major: AV / prefix matmuls move only 2
    columns into one PSUM tile (out[128q, 2] per query chunk).
  - PSUM-touching elementwise ops run only on Act/DVE (GPSIMD has no PSUM
    port); Pool gets SBUF-side mask multiplies fed by Act copies.
Timestamp compares stay exact fp32; padded slots have V=0 / t2=+inf so
padding is numerically inert.
"""

import numpy as np

T = 4096
D = 66
DA = 67                  # D + ones row (bias folding)
M = 4
PC = 128                 # rows per chunk (partition dim)
QCH = 16                 # query chunks per core
NSLOT = 33               # local key chunk slots per core
NP = 15                  # prefix-only slots (local 0..14)
NW = 18                  # window slots (local 15..32)
QW = QCH * PC            # 2048 queries per core
KW = NSLOT * PC          # 4224 local keys per core
NWPK = 4                 # wpack matrices (qA qB kA kB)
WPW = NWPK * D + 2 * DA  # wpack | M | Mq
ACTS_W = 2 * QW + KW + NW * PC   # hqa | hqb | hka | hkbw mega-tile width
TBIG = 1.0e30            # timestamp sentinel for padded keys


def _wslot_lbs(wc):
    """Query chunks whose window includes W-slot wc (local indices)."""
    lb0 = max(0, wc - 2)
    lb1 = min(QCH - 1, wc)
    return lb0, lb1


def _shard_host(x1, x2, x3, x4, wq, bq, wk, bk):
    """Build the 8 per-core input maps (host-side sharding/layout)."""
    xs = [np.asarray(x)[0, 0] for x in (x1, x2, x3, x4)]   # (4096, 66) each
    x1f = xs[0]
    wq = np.asarray(wq, np.float32)
    bq = np.asarray(bq, np.float32)
    wk = np.asarray(wk, np.float32)
    bk = np.asarray(bk, np.float32)

    def waug(w, b):
        return np.concatenate([w, b[None, :]], 0)          # (67, 66)

    wq3a = waug(wq[2], bq[2])
    wk3a = waug(wk[2], bk[2])
    Mm = (wk3a @ wq3a.T).astype(np.float16)                # (67, 67) = W6'Wq3'^T
    wpack = np.zeros((DA, WPW), np.float16)
    wpack[:, 0:D] = waug(wq[0], bq[0])
    wpack[:, D:2 * D] = waug(wq[1], bq[1])
    wpack[:, 2 * D:3 * D] = waug(wk[0], bk[0])
    wpack[:, 3 * D:4 * D] = waug(wk[1], bk[1])
    wpack[:, 4 * D:4 * D + DA] = Mm                        # M  (lhsT for P_q)
    wpack[:, 4 * D + DA:WPW] = Mm.T                        # Mq (lhsT for spre')
    ident16 = np.eye(PC, dtype=np.float16)
    onesrow = np.ones((1, ACTS_W), np.float16)

    in_maps = []
    for core in range(8):
        m, h = core // 2, core % 2
        xm = xs[m]
        g0 = 16 * (h - 1)            # local slot s <-> global chunk s + g0
        xqa = np.zeros((DA, QW), np.float16)
        xqa[:D] = x1f[2048 * h: 2048 * h + 2048, :].T.astype(np.float16)
        xqa[D] = 1.0
        xka = np.zeros((DA, KW), np.float16)
        xka[D] = 1.0
        lo_l = max(0, -g0)
        hi_l = min(NSLOT, 32 - g0)
        gl0 = (lo_l + g0) * PC
        gl1 = (hi_l + g0) * PC
        xka[:D, lo_l * PC: hi_l * PC] = xm[gl0:gl1].T.astype(np.float16)
        vt16 = np.zeros((PC, 2 * NSLOT), np.float16)
        t2c = np.full((PC, NSLOT), TBIG, np.float32)
        vreal = xm[gl0:gl1, 0:2].astype(np.float16).reshape(hi_l - lo_l, PC, 2)
        treal = xm[gl0:gl1, 65].astype(np.float32).reshape(hi_l - lo_l, PC)
        for s in range(lo_l, hi_l):
            vt16[:, 2 * s:2 * s + 2] = vreal[s - lo_l]
            t2c[:, s] = treal[s - lo_l]
        im = {
            "qin": np.concatenate([wpack, xqa], axis=1),   # [67, WPW+2048]
            "xka": xka,
            "vt16": vt16,
            "t2c": t2c,
            "t1r": np.ascontiguousarray(
                x1f[2048 * h: 2048 * h + 2048, 65][None, :]).astype(np.float32),
            "ident": ident16,
            "onesrow": onesrow,
        }
        in_maps.append(im)
    return in_maps


def _window_ok(x1, xs):
    """Check the bounded-rank-deviation assumption the device program needs."""
    t1 = np.asarray(x1)[0, 0, :, 65]
    for xm in xs:
        t2 = np.asarray(xm)[0, 0, :, 65]
        p = np.searchsorted(t2, t1, side="right")
        b = np.arange(32)
        if not (p[b * PC] >= (b - 1) * PC).all():
            return False
        if not (p[b * PC + PC - 1] <= (b + 2) * PC).all():
            return False
        # first 32 queries of each chunk take no keys from the next chunk
        # (needed for the 32-column upper-window trim)
        if not (p[b * PC + 31] <= (b + 1) * PC).all():
            return False
    return True


def _core_emulate(im):
    """Numpy emulation of the device program for one core (validation).

    Mirrors dtype rounding: fp16 at every SBUF materialization, fp32
    accumulation inside matmuls (PSUM).
    """
    f16 = np.float16
    f32 = np.float32

    def mm(a, b):
        return (a.astype(f32).T @ b.astype(f32))

    ws = [im["qin"][:, i * D:(i + 1) * D] for i in range(NWPK)]
    Mm = im["qin"][:, 4 * D:4 * D + DA]
    Mq = im["qin"][:, 4 * D + DA:WPW]
    xqa = im["qin"][:, WPW:]

    def aug(hfm):
        return np.concatenate([hfm, np.ones((1, hfm.shape[1]), f16)], 0)

    hqa = aug(np.maximum(mm(ws[0], xqa), 0).astype(f16))
    hqb = aug(np.maximum(mm(ws[1], hqa), 0).astype(f16))
    pq = mm(Mq, hqb).astype(f16)                           # (67, 2048) = M hqb

    hka = aug(np.maximum(mm(ws[2], im["xka"]), 0).astype(f16))
    wcols = slice(NP * PC, NSLOT * PC)
    hkbw = aug(np.maximum(mm(ws[3], hka[:, wcols]), 0).astype(f16))  # (67,2304)

    # P slots: token-major hb + G accumulation (for the shared prefix base)
    Gtot = np.zeros((DA, 2), f32)
    for pc in range(NP):
        cols = slice(pc * PC, (pc + 1) * PC)
        hb_tm = np.maximum(mm(hka[:, cols], ws[3]), 0).astype(f16)
        hb_tm = np.concatenate([hb_tm, np.ones((PC, 1), f16)], 1)
        v = im["vt16"][:, 2 * pc:2 * pc + 2]
        Gtot += mm(hb_tm, v)
    gt = Gtot.astype(f16)
    base = mm(Mm, gt).astype(f16)                          # (67, 2) = M^T Gt

    # W-slot G accumulation into per-pair prefixes, then Mq transform
    gw = np.zeros((QCH, DA, 2), f32)
    for wc in range(NP):
        hbkT = hkbw[:, wc * PC:(wc + 1) * PC].T            # (128, 67) f16
        v = im["vt16"][:, 2 * (NP + wc):2 * (NP + wc) + 2]
        g = mm(hbkT, v)                                    # (67, 2)
        for lb in range(wc + 1, QCH):
            gw[lb] += g
    sprep = np.zeros((QCH, DA, 2), f16)
    for lb in range(QCH):
        sprep[lb] = mm(Mm, gw[lb].astype(f16)).astype(f16)

    # scores + masks + AV (token-major out) + prefix/base closers
    t1 = im["t1r"][0]
    out = np.zeros((PC, 2 * QCH), f32)
    for wc in range(NW):
        lb0, lb1 = _wslot_lbs(wc)
        qcols = slice(lb0 * PC, (lb1 + 1) * PC)
        ps = mm(hkbw[:, wc * PC:(wc + 1) * PC], pq[:, qcols])
        t2 = im["t2c"][:, NP + wc]
        msc = (np.where(t1[None, qcols] >= t2[:, None], ps, 0)).astype(f16)
        if wc >= 2:
            msc[:, 0:32] = 0       # upper-window trim (device skips these)
        v = im["vt16"][:, 2 * (NP + wc):2 * (NP + wc) + 2]
        for lb in range(lb0, lb1 + 1):
            blk = msc[:, (lb - lb0) * PC:(lb - lb0 + 1) * PC]
            out[:, 2 * lb:2 * lb + 2] += mm(blk, v)
    for lb in range(QCH):
        hq = hqb[:, lb * PC:(lb + 1) * PC]
        if lb >= 1:
            out[:, 2 * lb:2 * lb + 2] += mm(hq, sprep[lb].astype(f32))
        out[:, 2 * lb:2 * lb + 2] += mm(hq, base.astype(f32))
    return out                                              # (128, 32)


def _combine(per_core_outs):
    full = np.zeros((T, 2), np.float32)
    for core, o in enumerate(per_core_outs):
        h = core % 2
        o = np.asarray(o, np.float32)                       # (128, 32)
        full[2048 * h: 2048 * h + 2048] += \
            o.reshape(PC, QCH, 2).transpose(1, 0, 2).reshape(QW, 2)
    return full[None, :, :]


def _numpy_fallback(x1, x2, x3, x4, wq, bq, wk, bk):
    """Exact dense fallback (used only if the window assumption fails)."""
    xs = [np.asarray(x)[0, 0].astype(np.float64) for x in (x1, x2, x3, x4)]

    def mlp(x, W, b):
        h = x
        for l in range(2):
            h = np.maximum(h @ W[l] + b[l], 0.0)
        return h @ W[2] + b[2]

    Q = mlp(xs[0], np.asarray(wq, np.float64), np.asarray(bq, np.float64))
    t1 = xs[0][:, 65]
    out = np.zeros((T, 2))
    for m in range(M):
        Km = mlp(xs[m], np.asarray(wk, np.float64), np.asarray(bk, np.float64))
        t2 = xs[m][:, 65]
        mask = t2[None, :] <= t1[:, None]
        A = (Q @ Km.T) * mask
        out += A @ xs[m][:, 0:2]
    return out[None].astype(np.float32)


# ---------------------------------------------------------------------------
# Bass device program
# ---------------------------------------------------------------------------

_NC_CACHE = {}


def _build_nc():
    import concourse.bacc as bacc
    import concourse.mybir as mybir
    import concourse.tile as tile

    f32 = mybir.dt.float32
    f16 = mybir.dt.float16
    AF = mybir.ActivationFunctionType
    ALU = mybir.AluOpType

    nc = bacc.Bacc("TRN2", target_bir_lowering=False, debug=False,
                   enable_asserts=False, num_devices=8)

    qin_d = nc.dram_tensor("qin", [DA, WPW + QW], f16, kind="ExternalInput")
    xka_d = nc.dram_tensor("xka", [DA, KW], f16, kind="ExternalInput")
    vt_d = nc.dram_tensor("vt16", [PC, 2 * NSLOT], f16, kind="ExternalInput")
    t2_d = nc.dram_tensor("t2c", [PC, NSLOT], f32, kind="ExternalInput")
    t1_d = nc.dram_tensor("t1r", [1, QW], f32, kind="ExternalInput")
    id_d = nc.dram_tensor("ident", [PC, PC], f16, kind="ExternalInput")
    ones_d = nc.dram_tensor("onesrow", [1, ACTS_W], f16, kind="ExternalInput")
    out_d = nc.dram_tensor("out", [PC, 2 * QCH], f32, kind="ExternalOutput")

    WCOL0 = NP * PC              # first column of W-slot region in xka/hka

    class Balancer:
        """Greedy Act/DVE picker for PSUM-sourced elementwise ops."""

        def __init__(self, nc):
            self.nc = nc
            self.acc = {"act": 0.0, "dve": 0.0, "pool": 0.0}
            # committed-but-not-yet-emitted load (masks pinned on DVE);
            # armed right before the score/mask phase starts
            self.future = {"act": 0.0, "dve": 0.0, "pool": 0.0}

        def _cost(self, e, free):
            if e == "act":
                return free * 0.8333 + 143.0
            if e == "dve":
                return free * 1.0417 + 125.0
            return free * 1.389 + 131.0

        def pick(self, free, engines=("act", "dve")):
            best = min(engines, key=lambda e: self.acc[e] + self.future[e]
                       + self._cost(e, free))
            self.acc[best] += self._cost(best, free)
            return best

        def relu(self, dst, src, free):
            if self.pick(free) == "act":
                self.nc.scalar.activation(dst, src, AF.Relu)
            else:
                self.nc.vector.tensor_scalar_max(dst, src, 0.0)

        def copy(self, dst, src, free):
            if self.pick(free) == "act":
                self.nc.scalar.copy(dst, src)
            else:
                self.nc.vector.tensor_copy(dst, src)

    with tile.TileContext(nc) as tc:
        with (
            tc.tile_pool(name="const", bufs=1) as cpool,
            tc.tile_pool(name="big", bufs=1) as bpool,
            tc.tile_pool(name="msc", bufs=18) as wpool,
            tc.tile_pool(name="ps_mlp", bufs=4, space="PSUM") as ps_mlp,
            tc.tile_pool(name="ps_sc", bufs=3, space="PSUM") as ps_sc,
            tc.tile_pool(name="ps_sm", bufs=1, space="PSUM") as ps_sm,
        ):
            bal = Balancer(nc)
            # ---- inputs; HWDGE dispatch order = emission order
            qin = cpool.tile([DA, WPW + QW], f16)
            xka = bpool.tile([DA, KW], f16)
            nc.sync.dma_start(qin[:, 0:WPW + 512], qin_d[:, 0:WPW + 512])
            # W-region of xka early: it gates the K-side critical chain
            nc.sync.dma_start(xka[:, WCOL0:KW], xka_d[:, WCOL0:KW])
            nc.sync.dma_start(qin[:, WPW + 512:], qin_d[:, WPW + 512:])
            nc.sync.dma_start(xka[:, 0:WCOL0], xka_d[:, 0:WCOL0])
            wsb = qin[:, 0:NWPK * D]
            m_sb = qin[:, 4 * D:4 * D + DA]
            mq_sb = qin[:, 4 * D + DA:WPW]
            xqa = qin[:, WPW:]
            vt16 = bpool.tile([PC, 2 * NSLOT], f16)
            nc.sync.dma_start(vt16[:], vt_d[:])
            t2c = bpool.tile([PC, NSLOT], f32)
            nc.sync.dma_start(t2c[:], t2_d[:])
            acts = bpool.tile([DA, ACTS_W], f16)
            nc.sync.dma_start(acts[D:DA, :], ones_d[:, 0:ACTS_W])
            t1b = bpool.tile([PC, QW], f32)
            for c0 in range(0, QW, 1024):
                nc.sync.dma_start(t1b[:, c0:c0 + 1024],
                                  t1_d[:, c0:c0 + 1024].broadcast_to((PC, 1024)))
            ident = cpool.tile([PC, PC], f16)
            nc.sync.dma_start(ident[:], id_d[:])

            # ---- activation views in the mega-tile (shared ones row)
            hqa = acts[:, 0:QW]
            hqb = acts[:, QW:2 * QW]
            hka = acts[:, 2 * QW:2 * QW + KW]
            hkbw = acts[:, 2 * QW + KW:ACTS_W]
            pq = bpool.tile([DA, QW], f16)         # M @ hqb'
            hbp = bpool.tile([PC, NP * DA], f16)   # token-major P-slot hb'
            hbkT = bpool.tile([PC, NP * DA], f16)  # token-major W-slot hb'
            ones_ap = hbp[:].rearrange("p (n c) -> p n c", c=DA)[:, :, D:DA]
            nc.gpsimd.memset(ones_ap, 1.0)

            sprep_sb = bpool.tile([DA, 2 * QCH], f16)
            gt_sb = bpool.tile([DA, 2], f16)
            base_sb = bpool.tile([DA, 2], f16)
            outT = bpool.tile([PC, 2 * QCH], f32)

            # Output PSUM bank.  HW note: a matmul accumulation group's
            # start=True clears the has_written bits of the WHOLE bank, so
            # groups sharing a bank must be emitted strictly one-at-a-time
            # (finished groups' data persists; only open groups break).
            outp = ps_sm.tile([PC, 2 * QCH], f32, tag="sm")

            def fm_block(dst, dst0, src, src0, w_ap, cw, func, name):
                ps = ps_mlp.tile([DA, 512], f32, tag="m", name=name)
                kd = w_ap.shape[0]
                od = w_ap.shape[1]
                nc.tensor.matmul(ps[:od, :cw], w_ap, src[:kd, src0:src0 + cw],
                                 start=True, stop=True)
                dst_ap = dst[0:od, dst0:dst0 + cw]
                if func == "relu":
                    bal.relu(dst_ap, ps[:od, :cw], cw)
                else:
                    bal.copy(dst_ap, ps[:od, :cw], cw)

            # ---------- state for the attention pipeline
            state = {"spre_upto": 0, "gw_upto": 0, "gt_done": False,
                     "pref_lb": 0, "out_half": 0, "av_done": 0}

            def emit_gw(wcs):
                """Per-pair GW prefix groups (pair lb = sum of G^W over
                wc <= lb-1), one consecutive accumulation group per pair so
                the PSUM bank only ever has one open group."""
                lo = state["gw_upto"] + 1
                hi = wcs[-1] + 1
                gch = ps_mlp.tile([DA, 2 * QCH], f32, tag="m",
                                  name=f"gch{lo}")
                for lb in range(lo, hi + 1):
                    for wc in range(lb):
                        nc.tensor.matmul(
                            gch[:, 2 * (lb - lo):2 * (lb - lo) + 2],
                            hbkT[:, wc * DA:(wc + 1) * DA],
                            vt16[:, 2 * (NP + wc):2 * (NP + wc) + 2],
                            start=(wc == 0), stop=(wc == lb - 1))
                gwc = bpool.tile([DA, 2 * QCH], f16, name=f"gwc{lo}",
                                 tag="gwc", bufs=2)
                nw2 = 2 * (hi - lo + 1)
                bal.copy(gwc[:, 0:nw2], gch[:, 0:nw2], nw2)
                state["gw_upto"] = hi
                return gwc, lo, hi

            def emit_tf(tf):
                """Mq-transform a finished GW chunk into prefix states."""
                gwc, lo, hi = tf
                nw2 = 2 * (hi - lo + 1)
                ps2 = ps_mlp.tile([DA, 2 * QCH], f32, tag="m", name=f"sp{lo}")
                nc.tensor.matmul(ps2[:, 0:nw2], m_sb, gwc[:, 0:nw2],
                                 start=True, stop=True)
                bal.copy(sprep_sb[:, 2 * lo:2 * lo + nw2],
                         ps2[:, 0:nw2], nw2)
                state["spre_upto"] = hi

            tr_batch = []          # wc indices awaiting transpose
            copied = []            # transposed+copied batches awaiting GW
            tfs = []               # GW chunks awaiting Mq transform

            def tr_step(final=False):
                """Advance the transpose -> GW -> Mq-transform pipeline by
                one stage (each stage lags so nothing waits inline)."""
                if tr_batch and (len(tr_batch) == 4 or final):
                    b0 = tr_batch[0]
                    n = len(tr_batch)
                    # stride 68 per slot: fp16 PSUM writes need 4B alignment
                    pst = ps_mlp.tile([PC, 4 * 68], f16, tag="m",
                                      name=f"tr{b0}")
                    for j, wc in enumerate(tr_batch):
                        nc.tensor.transpose(pst[:, j * 68:j * 68 + DA],
                                            hkbw[:, wc * PC:(wc + 1) * PC],
                                            ident[0:DA, 0:DA])
                    src3 = pst[:, :n * 68].rearrange(
                        "p (n c) -> p n c", c=68)[:, :, 0:DA]
                    dst3 = hbkT[:, b0 * DA:(b0 + n) * DA].rearrange(
                        "p (n c) -> p n c", c=DA)
                    bal.copy(dst3, src3, n * DA)
                    copied.append(list(tr_batch))
                    tr_batch.clear()
                if copied and (len(copied) > 1 or final):
                    tfs.append(emit_gw(copied.pop(0)))
                if tfs and (len(tfs) > 1 or final):
                    emit_tf(tfs.pop(0))

            def close_pairs(up_to_lb):
                """Emit whole output pairs: one consecutive accumulation
                group per pair (3 AV + prefix + base matmuls), so the outp
                bank never has two open groups."""
                if not state["gt_done"]:
                    return
                lim = min(up_to_lb, state["spre_upto"], QCH - 1)
                while state["pref_lb"] <= lim:
                    lb = state["pref_lb"]
                    for wc in (lb, lb + 1, lb + 2):
                        msc, lb0 = mscb[wc]
                        nc.tensor.matmul(
                            outp[:, 2 * lb:2 * lb + 2],
                            msc[:, (lb - lb0) * PC:(lb - lb0 + 1) * PC],
                            vt16[:, 2 * (NP + wc):2 * (NP + wc) + 2],
                            start=(wc == lb), stop=False)
                    mscb.pop(lb)      # pair lb was msc[lb]'s last reader
                    if lb >= 1:
                        nc.tensor.matmul(outp[:, 2 * lb:2 * lb + 2],
                                         hqb[:, lb * PC:(lb + 1) * PC],
                                         sprep_sb[:, 2 * lb:2 * lb + 2],
                                         start=False, stop=False)
                    nc.tensor.matmul(outp[:, 2 * lb:2 * lb + 2],
                                     hqb[:, lb * PC:(lb + 1) * PC],
                                     base_sb[:],
                                     start=False, stop=True)
                    state["pref_lb"] += 1
                while (state["out_half"] + 1) * 8 <= state["pref_lb"]:
                    hh = state["out_half"]
                    nc.vector.tensor_copy(outT[:, 16 * hh:16 * hh + 16],
                                          outp[:, 16 * hh:16 * hh + 16])
                    nc.sync.dma_start(out_d[:, 16 * hh:16 * hh + 16],
                                      outT[:, 16 * hh:16 * hh + 16])
                    state["out_half"] += 1

            mscb = {}

            def emit_wslot(wc):
                """Scores + mask + AV matmuls + csum transpose for W slot wc.
                Leading 32 columns of an upper-window block are provably
                fully masked (rank deviation < 96): skipped."""
                lb0, lb1 = _wslot_lbs(wc)
                ncol = (lb1 - lb0 + 1) * PC
                c0 = 32 if wc >= 2 else 0
                ps = ps_sc.tile([PC, 3 * PC], f32, tag="sc", name=f"sc{wc}")
                nc.tensor.matmul(ps[:, c0:ncol], hkbw[:, wc * PC:(wc + 1) * PC],
                                 pq[:, lb0 * PC + c0:(lb1 + 1) * PC],
                                 start=True, stop=True)
                msc = wpool.tile([PC, 3 * PC], f16, tag="msc", name=f"msc{wc}")
                if c0:
                    # upper-window AV uses the full 128 columns; the skipped
                    # leading 32 are zeroed here (Pool is idle, SBUF-legal)
                    nc.gpsimd.memset(msc[:, 0:c0], 0.0)
                t1ap = t1b[:, lb0 * PC + c0:(lb1 + 1) * PC]
                s_ap = t2c[:, NP + wc:NP + wc + 1]
                free = ncol - c0
                mcost = free * 1.0417 + 125.0
                bal.acc["dve"] += mcost
                bal.future["dve"] = max(0.0, bal.future["dve"] - mcost)
                nc.vector.scalar_tensor_tensor(
                    msc[:, c0:ncol], t1ap, s_ap, ps[:, c0:ncol],
                    ALU.is_ge, ALU.mult)
                mscb[wc] = (msc, lb0)
                if wc < NP:
                    tr_batch.append(wc)
                tr_step(final=(wc >= NP))
                # pairs lag 3 slots so their masks are done when they issue
                close_pairs(wc - 3)

            def kbp_batch(b0):
                ps = ps_mlp.tile([PC, 5 * D], f32, tag="m", name=f"kbp{b0}")
                for j, pc_ in enumerate(range(b0, b0 + 5)):
                    nc.tensor.matmul(ps[:, j * D:(j + 1) * D],
                                     hka[:, pc_ * PC:(pc_ + 1) * PC],
                                     wsb[:, 3 * D:4 * D],
                                     start=True, stop=True)
                dst = hbp[:].rearrange("p (n c) -> p n c", c=DA)[:, b0:b0 + 5, 0:D]
                src3 = ps[:].rearrange("p (n c) -> p n c", c=D)
                bal.relu(dst, src3, 5 * D)

            def g_and_base():
                gps = ps_mlp.tile([DA, 2], f32, tag="m", name="gps")
                for pc_ in range(NP):
                    nc.tensor.matmul(gps[:], hbp[:, pc_ * DA:(pc_ + 1) * DA],
                                     vt16[:, 2 * pc_:2 * pc_ + 2],
                                     start=(pc_ == 0), stop=(pc_ == NP - 1))
                nc.scalar.copy(gt_sb[:], gps[:])
                bal.acc["act"] += 145.0
                bps = ps_mlp.tile([DA, 2], f32, tag="m", name="bps")
                nc.tensor.matmul(bps[:], m_sb, gt_sb[:],
                                 start=True, stop=True)
                nc.scalar.copy(base_sb[:], bps[:])
                bal.acc["act"] += 145.0
                state["gt_done"] = True

            # job builders (all fm blocks are 512 wide)
            def QA(i): fm_block(hqa, 512 * i, xqa, 512 * i,
                                wsb[:, 0:D], 512, "relu", f"qa{i}")
            def QB(i): fm_block(hqb, 512 * i, hqa, 512 * i,
                                wsb[:, D:2 * D], 512, "relu", f"qb{i}")
            def PQ(i): fm_block(pq, 512 * i, hqb, 512 * i,
                                mq_sb, 512, "copy", f"pq{i}")
            def KAW(i):
                cw = min(512, NW * PC - 512 * i)
                fm_block(hka, WCOL0 + 512 * i, xka, WCOL0 + 512 * i,
                         wsb[:, 2 * D:3 * D], cw, "relu", f"kaw{i}")
            def KBW(i):
                cw = min(512, NW * PC - 512 * i)
                fm_block(hkbw, 512 * i, hka, WCOL0 + 512 * i,
                         wsb[:, 3 * D:4 * D], cw, "relu", f"kbw{i}")
            def KAP(i):
                cw = min(512, WCOL0 - 512 * i)
                fm_block(hka, 512 * i, xka, 512 * i,
                         wsb[:, 2 * D:3 * D], cw, "relu", f"kap{i}")
            def WS(a, b):
                for wc in range(a, b):
                    emit_wslot(wc)

            # W-critical chain first: the mask phase (DVE-serial) gates the
            # tail, so scores must start as early as possible.
            QA(0); KAW(0); QA(1); KAW(1); QA(2); KAW(2); QA(3); KAW(3)
            KAW(4)
            QB(0); KBW(0); QB(1); KBW(1); QB(2); KBW(2); QB(3); KBW(3)
            KBW(4)
            PQ(0); PQ(1); PQ(2); PQ(3)
            bal.future["dve"] = 8100.0
            # P-side work injected into the score/mask stream as filler
            WS(0, 2); KAP(0)
            WS(2, 4); KAP(1)
            WS(4, 6); KAP(2)
            WS(6, 8); KAP(3)
            WS(8, 10); kbp_batch(0)
            WS(10, 12); kbp_batch(5)
            WS(12, 14); kbp_batch(10)
            WS(14, 16); g_and_base()
            WS(16, NW)
            for _ in range(4):
                tr_step(final=True)
            close_pairs(QCH - 1)

    nc.compile()
    return nc


def _get_nc():
    if "nc" not in _NC_CACHE:
        _NC_CACHE["nc"] = _build_nc()
    return _NC_CACHE["nc"]


def kernel(x1, x2, x3, x4, wq, bq, wk, bk):
    xs = (x1, x2, x3, x4)
    if not _window_ok(x1, xs):
        return _numpy_fallback(x1, x2, x3, x4, wq, bq, wk, bk)
    in_maps = _shard_host(x1, x2, x3, x4, wq, bq, wk, bk)
    from concourse.bass_utils import run_bass_kernel_spmd
    nc = _get_nc()
    res = run_bass_kernel_spmd(nc, in_maps, list(range(8)))
    return _combine([r["out"] for r in res.results])
